# revision 3
# baseline (speedup 1.0000x reference)
"""Cosine-similarity loss kernel for Trainium2 (8 NeuronCores, data-parallel).

See kernel3.py for the architecture rationale.  v2 restructures for:
- 1 MiB DMAs ([128, 4096] tiles; 256 KiB transfers measured ~280 GB/s
  vs ~341 GB/s at 1 MiB).
- elementwise ops at [128, 2048] (better overhead amortization),
- psum blocks [65, 1024] with 4-deep buffering (8 banks exactly),
- tunable engine splits for squares and psum-evacuation copies.

Row pipes:  T16 blocks ship bf16 transposed (DVE tensor_tensor 2x /
ACT Square + TensorE ones-matmul reduce).  T8 blocks ship fp8
transposed (same pipe at fp8 rates, halved DMA).
"""

import numpy as np

N, D = 65536, 512
NCORES = 8
ROWS = N // NCORES          # 8192 rows per core
P = 128
CH = D // P                 # 4 d-chunks
SB = 4096                   # rows per DMA super-block
PB = 1024                   # rows per psum block
EW = 2048                   # rows per elementwise op

_cache = {}


def _spread(n: int, frac: float) -> list[bool]:
    return [int((i + 1) * frac + 0.5) - int(i * frac + 0.5) > 0 for i in range(n)]


def _build(
    reps: int = 1,
    hwloop: int = 1,
    sb8: int = 0,               # how many of the ROWS//SB super-blocks ship fp8
    f_sq_act: float = 0.55,     # fraction of square-ops on ACT (rest DVE)
    f_evac_act: float = 0.75,   # fraction of psum-evac copies on ACT
    io_bufs: int = 3,
    scr_bufs: int = 4,
    ps_bufs: int = 4,
):
    import concourse.bacc as bacc
    import concourse.tile as tile
    from concourse import mybir

    nc = bacc.Bacc("TRN2", target_bir_lowering=False, debug=False)
    f32 = mybir.dt.float32
    bf16 = mybir.dt.bfloat16
    f8 = mybir.dt.float8e4

    nsb = ROWS // SB
    sb16 = nsb - sb8
    tens = {}
    if sb16:
        tens["a16"] = nc.dram_tensor("a16", [D, sb16 * SB], bf16, kind="ExternalInput")
        tens["b16"] = nc.dram_tensor("b16", [D, sb16 * SB], bf16, kind="ExternalInput")
    if sb8:
        tens["a8"] = nc.dram_tensor("a8", [D, sb8 * SB], f8, kind="ExternalInput")
        tens["b8"] = nc.dram_tensor("b8", [D, sb8 * SB], f8, kind="ExternalInput")
    out = nc.dram_tensor("out", [P, 1], f32, kind="ExternalOutput")

    aps = {k: t.ap().rearrange("(c p) r -> c p r", p=P) for k, t in tens.items()}

    n_sq = 2 * nsb * CH * (SB // EW)
    sq_act = _spread(n_sq, f_sq_act)
    evac_act = _spread(nsb * (SB // PB), f_evac_act)

    with tile.TileContext(nc) as tc:
        with (
            tc.tile_pool(name="io", bufs=io_bufs) as io,
            tc.tile_pool(name="scr", bufs=scr_bufs) as scr,
            tc.tile_pool(name="psum", bufs=ps_bufs, space="PSUM") as psum,
            tc.tile_pool(name="stats", bufs=1) as stats,
            tc.tile_pool(name="sd", bufs=1, space="DRAM") as sd,
        ):
            sdram = sd.tile([3, ROWS], f32, tag="sdram")
            ones16 = stats.tile([P, 1], bf16, tag="ones16")
            nc.vector.memset(ones16, 1.0)

            def _pass_body():
              sqi = 0
              evi = 0
              for rep in range(reps):
                for sblk in range(nsb):
                    is8 = sblk >= sb16
                    dt_in = f8 if is8 else bf16
                    ak, bk = ("a8", "b8") if is8 else ("a16", "b16")
                    boff = (sblk - sb16 if is8 else sblk) * SB
                    # psum tiles for this super-block's PB-chunks
                    pss = [
                        psum.tile([65, PB], f32, tag="ps", name=f"ps{i}")
                        for i in range(SB // PB)
                    ]
                    for c in range(CH):
                        at = io.tile([P, SB], dt_in, tag="at")
                        bt = io.tile([P, SB], dt_in, tag="bt")
                        nc.sync.dma_start(out=at, in_=aps[ak][c, :, boff : boff + SB])
                        nc.sync.dma_start(out=bt, in_=aps[bk][c, :, boff : boff + SB])
                        start = c == 0
                        stop = c == CH - 1
                        for e in range(SB // EW):
                            es = slice(e * EW, (e + 1) * EW)
                            prod = scr.tile([P, EW], bf16, tag="prod")
                            sqa = scr.tile([P, EW], bf16, tag="sqa")
                            sqb = scr.tile([P, EW], bf16, tag="sqb")
                            nc.vector.tensor_mul(prod, at[:, es], bt[:, es])
                            for (t_in, t_out) in ((at, sqa), (bt, sqb)):
                                if sq_act[sqi % n_sq]:
                                    nc.scalar.activation(
                                        out=t_out, in_=t_in[:, es],
                                        func=mybir.ActivationFunctionType.Square,
                                    )
                                else:
                                    nc.vector.tensor_mul(
                                        t_out, t_in[:, es], t_in[:, es]
                                    )
                                sqi += 1
                            # matmuls: 512-wide sub-strips into the right
                            # psum block
                            for h in range(EW // 512):
                                g = e * EW + h * 512          # row offset in SB
                                ps = pss[g // PB]
                                po = g % PB
                                hs = slice(h * 512, (h + 1) * 512)
                                psl = slice(po, po + 512)
                                nc.tensor.matmul(
                                    ps[0:1, psl], ones16, prod[:, hs],
                                    start=start, stop=stop,
                                )
                                nc.tensor.matmul(
                                    ps[32:33, psl], ones16, sqa[:, hs],
                                    start=start, stop=stop,
                                )
                                nc.tensor.matmul(
                                    ps[64:65, psl], ones16, sqb[:, hs],
                                    start=start, stop=stop,
                                )
                    # evacuate each psum block
                    for pbi, ps in enumerate(pss):
                        stage = scr.tile([65, PB], f32, tag="stage")
                        if evac_act[evi % len(evac_act)]:
                            nc.scalar.copy(stage, ps[0:65, :])
                        else:
                            nc.vector.tensor_copy(stage, ps[0:65, :])
                        evi += 1
                        r0 = sblk * SB + pbi * PB
                        rsl = slice(r0, r0 + PB)
                        nc.sync.dma_start(out=sdram[0, rsl], in_=stage[0:1, :])
                        nc.sync.dma_start(out=sdram[1, rsl], in_=stage[32:33, :])
                        nc.sync.dma_start(out=sdram[2, rsl], in_=stage[64:65, :])

            if hwloop > 1:
                with tc.For_i(0, hwloop):
                    _pass_body()
            else:
                _pass_body()

            # epilogue: cos = ab * rsqrt(aa*bb); partial = sum over rows
            S = ROWS // P
            st_ab = stats.tile([P, S], f32, tag="st_ab")
            st_aa = stats.tile([P, S], f32, tag="st_aa")
            st_bb = stats.tile([P, S], f32, tag="st_bb")
            nc.sync.dma_start(out=st_ab, in_=sdram[0, :].rearrange("(p k) -> p k", p=P))
            nc.sync.dma_start(out=st_aa, in_=sdram[1, :].rearrange("(p k) -> p k", p=P))
            nc.sync.dma_start(out=st_bb, in_=sdram[2, :].rearrange("(p k) -> p k", p=P))
            denom = stats.tile([P, S], f32, tag="denom")
            nc.vector.tensor_mul(denom, st_aa, st_bb)
            nc.vector.reciprocal(denom, denom)
            nc.scalar.sqrt(denom, denom)
            cos = stats.tile([P, S], f32, tag="cos")
            nc.vector.tensor_mul(cos, st_ab, denom)
            cred = stats.tile([P, 1], f32, tag="cred")
            nc.vector.tensor_reduce(
                out=cred, in_=cos, axis=mybir.AxisListType.X, op=mybir.AluOpType.add
            )
            nc.sync.dma_start(out=out.ap(), in_=cred)

    nc.compile()
    return nc


def make_inputs(cxr: np.ndarray, ehr: np.ndarray, sb8: int):
    """Per-core transposed shards, bf16 super-blocks first then fp8."""
    import ml_dtypes

    bf16 = ml_dtypes.bfloat16
    f8 = ml_dtypes.float8_e4m3
    r16 = (ROWS // SB - sb8) * SB
    in_maps = []
    for c in range(NCORES):
        saT = np.ascontiguousarray(ehr[c * ROWS : (c + 1) * ROWS].T)  # [D, ROWS]
        sbT = np.ascontiguousarray(cxr[c * ROWS : (c + 1) * ROWS].T)
        m = {}
        if r16:
            m["a16"] = np.ascontiguousarray(saT[:, :r16].astype(bf16))
            m["b16"] = np.ascontiguousarray(sbT[:, :r16].astype(bf16))
        if r16 < ROWS:
            m["a8"] = np.ascontiguousarray(saT[:, r16:].astype(f8))
            m["b8"] = np.ascontiguousarray(sbT[:, r16:].astype(f8))
        in_maps.append(m)
    return in_maps


CFG = dict(sb8=0, f_sq_act=0.50, f_evac_act=0.75, io_bufs=5, scr_bufs=6)


def kernel(cxr: np.ndarray, ehr: np.ndarray) -> np.ndarray:
    from concourse.bass_utils import run_bass_kernel_spmd

    cxr = np.asarray(cxr)
    ehr = np.asarray(ehr)
    assert cxr.shape == (N, D) and ehr.shape == (N, D)

    key = tuple(sorted(CFG.items()))
    if key not in _cache:
        _cache[key] = _build(**CFG)
    nc = _cache[key]

    in_maps = make_inputs(cxr, ehr, CFG["sb8"])
    res = run_bass_kernel_spmd(nc, in_maps, core_ids=list(range(NCORES)))
    total = np.float64(0.0)
    for r in res.results:
        total += r["out"].astype(np.float64).sum()
    return np.float32(1.0 - total / N)
